# revision 1
# baseline (speedup 1.0000x reference)
"""Causal squeeze-excite 1d on 8 TRN2 NeuronCores.

Reference computation (per batch b):
    y = causal_ema(x)                      # y[t] = (1-a) y[t-1] + a x[t], y[0] = x[0]
    h = relu(w1 @ y[:, t] + b1)            # (32,)  per time step
    g = sigmoid(w2 @ h + b2)               # (512,) per time step
    out[:, t] = x[:, t] * g

Sharding: data-parallel over batch. Core i gets x[2i:2i+2]; the tiny MLP
weights are replicated.

Key algebraic identity: the EMA is linear with channel-independent
coefficients, so it commutes with the channel projection:
    w1 @ ema(x) == ema(w1 @ x).
The kernel therefore projects first (p = (a*w1) @ x on the TensorEngine,
contracting C=512) and scans p — a [32, T] sequence — instead of the
[512, T] input. That cuts the DVE scan (the only engine that can run
tensor_tensor_scan, at 2 cyc/col) by 4x, from ~78us to ~17us per core.

Per-core pipeline, chunked along T with all streams double-buffered:
  - one coalesced DMA loads a [128, 4*Tc] x chunk (all 4 channel blocks);
  - mm1 (float32r) accumulates the 4 channel blocks into PSUM p tiles;
  - DVE tensor_tensor_scan consumes p straight out of PSUM
    (u_t = (1-a) u_{t-1} + p_t, rescaled so y = a*u is folded into w1);
  - relu(+b1) -> h (float32r), mm2 (float32r) -> PSUM, sigmoid(+b2) -> G;
  - gate multiply x*G into a separate output tile (DVE/GPSIMD split);
  - one coalesced DMA stores the [128, 4*Tc] output chunk.
float32r matmuls run 4x faster than fp32 on the PE; producer tiles are
typed float32r so the walrus verifier accepts them as rounded inputs.
"""

import numpy as np
from contextlib import ExitStack

import concourse.bass as bass
import concourse.bacc as bacc
import concourse.tile as tile
import concourse.mybir as mybir
from concourse.bass_utils import run_bass_kernel_spmd

F32 = mybir.dt.float32
F32R = mybir.dt.float32r

N_CORES = 8
B, C, T = 16, 512, 4096
CSQ = 32          # squeeze dim
P = 128           # SBUF partitions


def build_nc(B_loc, cw, C_=C, T_=T, Tc=1024, TS=512):
    """Build the per-core Bass program. Shapes are compile-time constants."""
    d = 1.0 - 1.0 / cw
    NCB = C_ // P      # channel blocks
    NTH = T_ // Tc     # time chunks
    NTS = Tc // TS     # matmul sub-blocks per chunk

    nc = bacc.Bacc(trn_type="TRN2")
    x = nc.declare_dram_parameter("x", [B_loc, C_, T_], F32R, isOutput=False)
    w1sT = nc.declare_dram_parameter("w1sT", [C_, CSQ], F32R, isOutput=False)
    b1 = nc.declare_dram_parameter("b1", [CSQ, 1], F32, isOutput=False)
    w2T = nc.declare_dram_parameter("w2T", [CSQ, C_], F32R, isOutput=False)
    b2 = nc.declare_dram_parameter("b2", [P, NCB], F32, isOutput=False)
    out = nc.declare_dram_parameter("out", [B_loc, C_, T_], F32, isOutput=True)

    with ExitStack() as ctx:
        tc = ctx.enter_context(tile.TileContext(nc))
        const = ctx.enter_context(tc.tile_pool(name="const", bufs=1))
        # One coalesced x tile per (b, th) chunk: [P, NCB*Tc], so a single
        # max-efficiency DMA covers the whole chunk; 4 bufs keep the DMA
        # rings streaming while compute runs.
        xpool = ctx.enter_context(tc.tile_pool(name="xp", bufs=5))
        opool = ctx.enter_context(tc.tile_pool(name="op", bufs=3))
        upool = ctx.enter_context(tc.tile_pool(name="up", bufs=6))
        hpool = ctx.enter_context(tc.tile_pool(name="hp", bufs=6))
        gpool = ctx.enter_context(tc.tile_pool(name="gp", bufs=6))
        cpool = ctx.enter_context(tc.tile_pool(name="cp", bufs=4))
        php = ctx.enter_context(tc.tile_pool(name="php", bufs=4, space="PSUM"))
        pgp = ctx.enter_context(tc.tile_pool(name="pgp", bufs=4, space="PSUM"))

        dconst = const.tile([CSQ, TS], F32, tag="dconst")
        nc.vector.memset(dconst[:], d)
        w1_t = []
        for cb in range(NCB):
            wt = const.tile([P, CSQ], F32R, tag=f"w1_{cb}")
            nc.sync.dma_start(wt[:], w1sT[cb * P:(cb + 1) * P, :])
            w1_t.append(wt)
        b1_t = const.tile([CSQ, 1], F32, tag="b1")
        nc.sync.dma_start(b1_t[:], b1[:])
        w2_t = const.tile([CSQ, C_], F32R, tag="w2")
        nc.sync.dma_start(w2_t[:], w2T[:])
        b2_t = const.tile([P, NCB], F32, tag="b2")
        nc.sync.dma_start(b2_t[:], b2[:])

        # DRAM views with channel blocks folded into the free dim:
        # [B, P, NCB, T] so one DMA moves a whole (b, th) chunk.
        xv = x.rearrange("b (cb p) t -> b p cb t", p=P)
        ov = out.rearrange("b (cb p) t -> b p cb t", p=P)

        # Interleave the independent batch streams: the scan chain serializes
        # along th within one b, so alternating b keeps a second independent
        # stream in flight.
        carries = {b: None for b in range(B_loc)}
        mul_idx = 0
        for th in range(NTH):
            for b in range(B_loc):
                xt = xpool.tile([P, NCB * Tc], F32R, tag="x")
                # Per-cb loads into the coalesced tile: mm1 for a block can
                # start as soon as that block lands, and the first chunk's
                # pipeline fill shrinks by ~3/4 of the load time.
                for cb in range(NCB):
                    nc.sync.dma_start(
                        xt[:, cb * Tc:(cb + 1) * Tc],
                        xv[b, :, cb, th * Tc:(th + 1) * Tc])
                ot = opool.tile([P, NCB * Tc], F32, tag="o")
                gts = [gpool.tile([P, Tc], F32, tag="g", name=f"g{cb}")
                       for cb in range(NCB)]
                for ts in range(NTS):
                    # p = (a*w1) @ x, contracting C across the 4 channel
                    # blocks into one PSUM tile.
                    ph = php.tile([CSQ, TS], F32, tag="ph")
                    for cb in range(NCB):
                        nc.tensor.matmul(
                            ph[:], w1_t[cb][:],
                            xt[:, cb * Tc + ts * TS:cb * Tc + (ts + 1) * TS],
                            start=(cb == 0), stop=(cb == NCB - 1))
                    # EMA scan straight out of PSUM: u_t = d*u_{t-1} + p_t.
                    ut = upool.tile([CSQ, TS], F32R, tag="u")
                    if th == 0 and ts == 0:
                        init = cpool.tile([CSQ, 1], F32, tag="c")
                        nc.scalar.mul(init[:], ph[:, 0:1], float(cw))
                        init_ap = init[:]
                    else:
                        # Chain straight off the previous u tile: DVE-to-DVE
                        # stays in engine order, no cross-engine hop.
                        init_ap = carries[b][:, TS - 1:TS]
                    nc.vector.tensor_tensor_scan(
                        ut[:], dconst[:], ph[:], init_ap,
                        mybir.AluOpType.mult, mybir.AluOpType.add)
                    carries[b] = ut
                    ht = hpool.tile([CSQ, TS], F32R, tag="h")
                    nc.scalar.activation(
                        ht[:], ut[:], mybir.ActivationFunctionType.Relu,
                        bias=b1_t[:])
                    for cb in range(NCB):
                        pg = pgp.tile([P, TS], F32, tag="pg")
                        nc.tensor.matmul(
                            pg[:], w2_t[:, cb * P:(cb + 1) * P],
                            ht[:], start=True, stop=True)
                        nc.scalar.activation(
                            gts[cb][:, ts * TS:(ts + 1) * TS], pg[:],
                            mybir.ActivationFunctionType.Sigmoid,
                            bias=b2_t[:, cb:cb + 1])
                for cb in range(NCB):
                    # Gate multiply into the output tile. DVE does ~1.4us per
                    # [P, Tc] op, GPSIMD ~3us but is otherwise idle; an
                    # 18/14 split balances both near ~43us. Per-cb stores
                    # start streaming as soon as that block's multiply lands.
                    sl = slice(cb * Tc, (cb + 1) * Tc)
                    mul_eng = nc.vector if mul_idx % 16 < 9 else nc.gpsimd
                    mul_idx += 1
                    mul_eng.tensor_mul(ot[:, sl], xt[:, sl], gts[cb][:])
                    nc.scalar.dma_start(
                        ov[b, :, cb, th * Tc:(th + 1) * Tc], ot[:, sl])
    nc.compile()
    return nc


def make_in_maps(x, w1, b1, w2, b2, cw, n_cores=N_CORES):
    """Host-side shard + weight prep. Returns per-core input maps."""
    a = 1.0 / cw
    w1sT = np.ascontiguousarray((w1 * a).T, dtype=np.float32)      # [C, CSQ]
    b1c = np.ascontiguousarray(b1.reshape(-1, 1), dtype=np.float32)
    w2T = np.ascontiguousarray(w2.T, dtype=np.float32)             # [CSQ, C]
    ncb = w2.shape[0] // P
    b2c = np.ascontiguousarray(b2.reshape(ncb, P).T, dtype=np.float32)  # [P, NCB]
    b_loc = x.shape[0] // n_cores
    return [
        {
            "x": np.ascontiguousarray(x[i * b_loc:(i + 1) * b_loc], dtype=np.float32),
            "w1sT": w1sT, "b1": b1c, "w2T": w2T, "b2": b2c,
        }
        for i in range(n_cores)
    ]


_NC_CACHE = {}


def kernel(x, w1, b1, w2, b2, context_window):
    cw = int(context_window)
    x = np.asarray(x)
    key = (cw, x.shape)
    if key not in _NC_CACHE:
        _NC_CACHE[key] = build_nc(x.shape[0] // N_CORES, cw)
    nc = _NC_CACHE[key]
    in_maps = make_in_maps(
        np.asarray(x), np.asarray(w1), np.asarray(b1),
        np.asarray(w2), np.asarray(b2), cw)
    res = run_bass_kernel_spmd(nc, in_maps, core_ids=list(range(N_CORES)))
    return np.concatenate([r["out"] for r in res.results], axis=0)



# revision 2
# speedup vs baseline: 1.0207x; 1.0207x over previous
"""Causal squeeze-excite 1d on 8 TRN2 NeuronCores — fp16-I/O version.

Reference computation (per batch b):
    y = causal_ema(x)                      # y[t] = (1-a) y[t-1] + a x[t], y[0] = x[0]
    h = relu(w1 @ y[:, t] + b1)            # (32,)  per time step
    g = sigmoid(w2 @ h + b2)               # (512,) per time step
    out[:, t] = x[:, t] * g

Sharding: data-parallel over batch; core i gets x[2i:2i+2].

The kernel is DMA-bound: it must stream all of x in and out once. fp32
I/O costs 33.5 MB/core (~94us at the 358 GB/s HBM-per-core limit); fp16
I/O costs 16.8 MB (~47us floor) and the fp16 rounding error (~8e-4 max
rel err, host-validated) is far inside the 2e-3 gate, so x is converted
to fp16 on the host and the output is returned as fp16.

Algebra: the EMA commutes with the channel projection, so the kernel
scans p = (a*w1) @ x — a [32, T] sequence — instead of the [512, T]
input (w1 @ ema(x) == ema(w1 @ x)).

Layout trick: both batches AND two copies are stacked in the partition
dim: P4[128, TS] = [p_b0; p_b0; p_b1; p_b1]. The DVE scan's cost is
per-column (2 cyc/col) regardless of partitions, so one scan covers
both batches (8.5us/core instead of 17). The duplicate copies make the
four mm2 matmuls per sub-chunk row-tiled (K=32 row groups 0..3), so
pairs run concurrently on the PE, and each (b0,b1) pair of mm2 outputs
lands in one 2-bank PSUM region [128, 1024] that a single sigmoid
activation consumes (FD=1024 amortizes the ACT per-instruction
overhead: 35us total instead of 46).

mm1 writes all 128 partitions per matmul (weights duplicated/zero
padded in M) so the 8 accumulating matmuls form ONE PSUM group — two
groups in one bank would race on the has_written bulk-clear.

relu(+b1) runs on the DVE (tensor_scalar add+max), keeping the scalar
engine free for sigmoid. The gate multiply splits DVE/GPSIMD. All DMA
goes through the sync queue (HWDGE) to keep ACT clean.

Per-core engine budget: DMA ~47us (bound), ACT ~36, PE ~27, DVE ~25,
GPSIMD ~26, sync ~27.
"""

import numpy as np
from contextlib import ExitStack

import concourse.bass as bass
import concourse.bacc as bacc
import concourse.tile as tile
import concourse.mybir as mybir
from concourse.bass_utils import run_bass_kernel_spmd

F16 = mybir.dt.float16
F32 = mybir.dt.float32

N_CORES = 8
B, C, T = 16, 512, 4096
B_LOC = B // N_CORES     # 2
CSQ = 32                 # squeeze dim
P = 128                  # SBUF partitions
NCB = C // P             # 4 channel blocks
TS = 512                 # mm2/sigmoid/mul granularity (PSUM bank = 512 f32)
TSC = 1024               # mm1/scan/relu granularity (fp16 moving max)
TCD = 1024               # DMA window (time)
SPC = TCD // TSC         # scan chunks per DMA window
NTD = T // TCD           # DMA windows
NSC = T // TSC           # scan chunks


def build_nc(b_loc, cw):
    assert b_loc == B_LOC
    d = 1.0 - 1.0 / cw
    nc = bacc.Bacc(trn_type="TRN2")
    x = nc.declare_dram_parameter("x", [B_LOC, C, T], F16, isOutput=False)
    w1z = nc.declare_dram_parameter("w1z", [C, 2 * P], F16, isOutput=False)
    w2z = nc.declare_dram_parameter("w2z", [P, 2 * P], F16, isOutput=False)
    b1z = nc.declare_dram_parameter("b1z", [P, 1], F32, isOutput=False)
    b2z = nc.declare_dram_parameter("b2z", [P, NCB], F32, isOutput=False)
    out = nc.declare_dram_parameter("out", [B_LOC, C, T], F16, isOutput=True)

    with ExitStack() as ctx:
        tc = ctx.enter_context(tile.TileContext(nc))
        # PSUM: pg tiles are 2 banks each and must stay bank-aligned, so
        # this pool is created first (pools stack from psum bank 0).
        pgp = ctx.enter_context(tc.tile_pool(name="pgp", bufs=2, space="PSUM"))
        php = ctx.enter_context(tc.tile_pool(name="php", bufs=2, space="PSUM"))
        const = ctx.enter_context(tc.tile_pool(name="const", bufs=1))
        xp = ctx.enter_context(tc.tile_pool(name="xp", bufs=NTD))
        op = ctx.enter_context(tc.tile_pool(name="op", bufs=3))
        up = ctx.enter_context(tc.tile_pool(name="up", bufs=4))
        hp = ctx.enter_context(tc.tile_pool(name="hp", bufs=4))
        gp = ctx.enter_context(tc.tile_pool(name="gp", bufs=6))
        cp = ctx.enter_context(tc.tile_pool(name="cp", bufs=1))

        # DRAM views with channel blocks split out: c = cb*128 + p.
        xv = x.rearrange("b (cb p) t -> b p cb t", p=P)
        ov = out.rearrange("b (cb p) t -> b p cb t", p=P)
        ovt = out.rearrange("b (cb p) t -> p cb b t", p=P)

        # Load order tuned for the pipeline fill: the first x window
        # (split per-batch so the transfers parallelize across DMA
        # queues) and the mm1 weights go first, with as few sync-queue
        # dispatches as possible — each dma_start costs the sync
        # sequencer ~750 ns, and compute can't start until these land.
        xtiles = {}
        xt = xp.tile([P, NCB, B_LOC, TCD], F16, tag="x", name="x0")
        for b in range(B_LOC):
            nc.sync.dma_start(xt[:, :, b, :], xv[b, :, :, 0:TCD])
        xtiles[0] = xt
        w1all = const.tile([P, NCB, 2 * P], F16, tag="w1")
        nc.sync.dma_start(
            w1all[:], w1z.rearrange("(cb p) m -> p cb m", p=P))
        w1t = {
            (b, cb): w1all[:, cb, b * P:(b + 1) * P]
            for b in range(B_LOC) for cb in range(NCB)
        }
        w2t = const.tile([P, 2 * P], F16, tag="w2")
        nc.sync.dma_start(w2t[:], w2z[:])
        b1t = const.tile([P, 1], F32, tag="b1")
        nc.sync.dma_start(b1t[:], b1z[:])
        b2t = const.tile([P, NCB], F32, tag="b2")
        nc.sync.dma_start(b2t[:], b2z[:])
        dconst = const.tile([P, TSC], F32, tag="dconst")
        nc.vector.memset(dconst[:], d)
        # Remaining x windows, all issued upfront (8 MB total fits
        # SBUF) so the DMA engines stream back-to-back.
        for td in range(1, NTD):
            xt = xp.tile([P, NCB, B_LOC, TCD], F16, tag="x", name=f"x{td}")
            for b in range(B_LOC):
                nc.sync.dma_start(
                    xt[:, :, b, :], xv[b, :, :, td * TCD:(td + 1) * TCD])
            xtiles[td] = xt

        def mm1(s):
            # p4 = [p_b0; p_b0; p_b1; p_b1] over a TSC-wide window. A
            # matmul output must fit one PSUM bank (512 f32), so the
            # 2-bank ph tile is filled by two interleaved 8-matmul
            # groups (one per bank — has_written bits are per bank).
            # Adjacent matmuls share the same weights.
            td, off = divmod(s, SPC)
            off *= TSC
            ph = php.tile([P, TSC], F32, tag="ph", name=f"ph{s}")
            k = 0
            for cb in range(NCB):
                for b in range(B_LOC):
                    for hf in range(TSC // TS):
                        o = off + hf * TS
                        nc.tensor.matmul(
                            ph[:, hf * TS:(hf + 1) * TS], w1t[b, cb],
                            xtiles[td][:, cb, b, o:o + TS],
                            start=(k <= 1), stop=(k >= 2 * NCB * 2 - 2))
                        k += 1
            return ph

        # Software-pipelined main loop. Per iteration sc we emit, in order:
        #   - mm1 for sc+1 (PE queue: ahead of sc's mm2s, so the scan
        #     feed never stalls on PE head-of-line blocking)
        #   - scan + relu for sc+1 (DVE queue: ahead of sc's gate-muls,
        #     which block on ACT sigmoids — otherwise the scan chain
        #     inherits the whole sigmoid cadence each chunk)
        #   - mm2 / sigmoid / gate-mul block for sc
        def scan_relu(sc, ph):
            if sc == 0:
                init = cp.tile([P, 1], F32, tag="c")
                nc.scalar.mul(init[:], ph[:, 0:1], float(cw))
                init_ap = init[:]
            else:
                init_ap = uts[sc - 1][:, TSC - 1:TSC]
            ut = up.tile([P, TSC], F16, tag="u", name=f"u{sc}")
            nc.vector.tensor_tensor_scan(
                ut[:], dconst[:], ph[:], init_ap,
                mybir.AluOpType.mult, mybir.AluOpType.add)
            uts[sc] = ut
            ht = hp.tile([P, TSC], F16, tag="h", name=f"h{sc}")
            nc.vector.tensor_scalar(
                ht[:], ut[:], b1t[:], 0.0,
                mybir.AluOpType.add, mybir.AluOpType.max)
            return ht

        otiles = {}
        uts = {}
        hts = {}
        ph = mm1(0)
        hts[0] = scan_relu(0, ph)
        for sc in range(NSC):
            td = sc * TSC // TCD
            coff = sc * TSC - td * TCD
            if coff == 0:
                otiles[td] = op.tile(
                    [P, NCB, B_LOC, TCD], F16, tag="o", name=f"o{td}")
            xt = xtiles[td]
            ot = otiles[td]

            if sc + 1 < NSC:
                ph = mm1(sc + 1)
                hts[sc + 1] = scan_relu(sc + 1, ph)
            ht = hts.pop(sc)

            # Per (half, channel block): row-tiled mm2 pair -> 2-bank PSUM
            # region, one sigmoid over [b0|b1], then two gate multiplies.
            for hf in range(TSC // TS):
                hoff = hf * TS
                off = coff + hoff
                for cb in range(NCB):
                    pg = pgp.tile([P, 2 * TS], F32, tag="pg")
                    rg0 = cb & 1
                    rg1 = 2 + (cb & 1)
                    half = cb >> 1
                    nc.tensor.matmul(
                        pg[:, 0:TS],
                        w2t[32 * rg0:32 * rg0 + 32, P * half:P * (half + 1)],
                        ht[32 * rg0:32 * rg0 + 32, hoff:hoff + TS],
                        start=True, stop=True, tile_position=(32 * rg0, 0))
                    nc.tensor.matmul(
                        pg[:, TS:2 * TS],
                        w2t[32 * rg1:32 * rg1 + 32, P * half:P * (half + 1)],
                        ht[32 * rg1:32 * rg1 + 32, hoff:hoff + TS],
                        start=True, stop=True, tile_position=(32 * rg1, 0))
                    gt = gp.tile([P, 2 * TS], F16, tag="g")
                    nc.scalar.activation(
                        gt[:], pg[:], mybir.ActivationFunctionType.Sigmoid,
                        bias=b2t[:, cb:cb + 1])
                    # All gate-muls on the DVE: gpsimd shares the DVE's
                    # SBUF port; a concurrent gpsimd op slows DVE ~3x.
                    for b in range(B_LOC):
                        nc.vector.tensor_mul(
                            ot[:, cb, b, off:off + TS],
                            xt[:, cb, b, off:off + TS],
                            gt[:, TS * b:TS * (b + 1)])
                    if sc == NSC - 1 and hf == TSC // TS - 1:
                        # Tail: store per cb as each mul lands.
                        nc.sync.dma_start(
                            ovt[:, cb, :, td * TCD:(td + 1) * TCD],
                            ot[:, cb, :, :])

            if coff + TSC == TCD and td != NTD - 1:
                for b in range(B_LOC):
                    nc.sync.dma_start(
                        ov[b, :, :, td * TCD:(td + 1) * TCD], ot[:, :, b, :])
    nc.compile()
    return nc


def make_in_maps(x, w1, b1, w2, b2, cw, n_cores=N_CORES):
    """Host-side shard + weight prep. Returns per-core input maps."""
    a = 1.0 / cw
    w1s16 = np.ascontiguousarray((w1 * a).T).astype(np.float16)    # [C, CSQ]
    w1zz = np.zeros((C, 2 * P), dtype=np.float16)
    # b0 matmul fills out-partitions 0-63 (two copies), b1 fills 64-127.
    w1zz[:, 0:CSQ] = w1s16
    w1zz[:, CSQ:2 * CSQ] = w1s16
    w1zz[:, P + 2 * CSQ:P + 3 * CSQ] = w1s16
    w1zz[:, P + 3 * CSQ:P + 4 * CSQ] = w1s16
    w2T16 = np.ascontiguousarray(w2.T).astype(np.float16)          # [CSQ, C]
    w2zz = np.zeros((P, 2 * P), dtype=np.float16)
    for rg in range(4):
        for hf in range(2):
            cb = 2 * hf + (rg & 1)
            w2zz[CSQ * rg:CSQ * (rg + 1), P * hf:P * (hf + 1)] = \
                w2T16[:, cb * P:(cb + 1) * P]
    b1zz = np.ascontiguousarray(
        np.tile(np.asarray(b1, np.float32), 4).reshape(P, 1))
    b2zz = np.ascontiguousarray(
        np.asarray(b2, np.float32).reshape(NCB, P).T)
    b_loc = x.shape[0] // n_cores
    x16 = np.asarray(x).astype(np.float16)
    return [
        {
            "x": np.ascontiguousarray(x16[i * b_loc:(i + 1) * b_loc]),
            "w1z": w1zz, "w2z": w2zz, "b1z": b1zz, "b2z": b2zz,
        }
        for i in range(n_cores)
    ]


_NC_CACHE = {}


def kernel(x, w1, b1, w2, b2, context_window):
    cw = int(context_window)
    x = np.asarray(x)
    key = (cw, x.shape)
    if key not in _NC_CACHE:
        _NC_CACHE[key] = build_nc(x.shape[0] // N_CORES, cw)
    nc = _NC_CACHE[key]
    in_maps = make_in_maps(
        x, np.asarray(w1), np.asarray(b1), np.asarray(w2), np.asarray(b2), cw)
    res = run_bass_kernel_spmd(nc, in_maps, core_ids=list(range(N_CORES)))
    return np.concatenate(
        [r["out"] for r in res.results], axis=0).astype(np.float32)


# revision 3
# speedup vs baseline: 1.0352x; 1.0143x over previous
"""Causal squeeze-excite 1d on 8 TRN2 NeuronCores — fp16-I/O version.

Reference computation (per batch b):
    y = causal_ema(x)                      # y[t] = (1-a) y[t-1] + a x[t], y[0] = x[0]
    h = relu(w1 @ y[:, t] + b1)            # (32,)  per time step
    g = sigmoid(w2 @ h + b2)               # (512,) per time step
    out[:, t] = x[:, t] * g

Sharding: data-parallel over batch; core i gets x[2i:2i+2].

The kernel is DMA-bound: it must stream all of x in and out once. fp32
I/O costs 33.5 MB/core (~94us at the 358 GB/s HBM-per-core limit); fp16
I/O costs 16.8 MB (~47us floor) and the fp16 rounding error (~8e-4 max
rel err, host-validated) is far inside the 2e-3 gate, so x is converted
to fp16 on the host and the output is returned as fp16.

Algebra: the EMA commutes with the channel projection, so the kernel
scans p = (a*w1) @ x — a [32, T] sequence — instead of the [512, T]
input (w1 @ ema(x) == ema(w1 @ x)).

Layout trick: both batches AND two copies are stacked in the partition
dim: P4[128, TS] = [p_b0; p_b0; p_b1; p_b1]. The DVE scan's cost is
per-column (2 cyc/col) regardless of partitions, so one scan covers
both batches (8.5us/core instead of 17). The duplicate copies make the
four mm2 matmuls per sub-chunk row-tiled (K=32 row groups 0..3), so
pairs run concurrently on the PE, and each (b0,b1) pair of mm2 outputs
lands in one 2-bank PSUM region [128, 1024] that a single sigmoid
activation consumes (FD=1024 amortizes the ACT per-instruction
overhead: 35us total instead of 46).

mm1 writes all 128 partitions per matmul (weights duplicated/zero
padded in M) so the 8 accumulating matmuls form ONE PSUM group — two
groups in one bank would race on the has_written bulk-clear.

relu(+b1) runs on the DVE (tensor_scalar add+max), keeping the scalar
engine free for sigmoid. The gate multiply splits DVE/GPSIMD. All DMA
goes through the sync queue (HWDGE) to keep ACT clean.

Per-core engine budget: DMA ~47us (bound), ACT ~36, PE ~27, DVE ~25,
GPSIMD ~26, sync ~27.
"""

import numpy as np
from contextlib import ExitStack

import concourse.bass as bass
import concourse.bacc as bacc
import concourse.tile as tile
import concourse.mybir as mybir
from concourse.bass_utils import run_bass_kernel_spmd

F16 = mybir.dt.float16
F32 = mybir.dt.float32

N_CORES = 8
B, C, T = 16, 512, 4096
B_LOC = B // N_CORES     # 2
CSQ = 32                 # squeeze dim
P = 128                  # SBUF partitions
NCB = C // P             # 4 channel blocks
TS = 512                 # mm2/sigmoid/mul granularity (PSUM bank = 512 f32)
TSC = 512                # mm1/scan/relu granularity
TCD = 1024               # DMA window (time)
SPC = TCD // TSC         # scan chunks per DMA window
NTD = T // TCD           # DMA windows
NSC = T // TSC           # scan chunks


def build_nc(b_loc, cw):
    assert b_loc == B_LOC
    d = 1.0 - 1.0 / cw
    nc = bacc.Bacc(trn_type="TRN2")
    x = nc.declare_dram_parameter("x", [B_LOC, C, T], F16, isOutput=False)
    w1z = nc.declare_dram_parameter("w1z", [P, NCB * 2 * P], F16,
                                    isOutput=False)
    w2z = nc.declare_dram_parameter("w2z", [P, 2 * P], F16, isOutput=False)
    b1z = nc.declare_dram_parameter("b1z", [P, 1], F32, isOutput=False)
    b2z = nc.declare_dram_parameter("b2z", [P, NCB], F32, isOutput=False)
    out = nc.declare_dram_parameter("out", [B_LOC, C, T], F16, isOutput=True)

    with ExitStack() as ctx:
        tc = ctx.enter_context(tile.TileContext(nc))
        # PSUM: pg tiles are 2 banks each and must stay bank-aligned, so
        # this pool is created first (pools stack from psum bank 0).
        pgp = ctx.enter_context(tc.tile_pool(name="pgp", bufs=2, space="PSUM"))
        php = ctx.enter_context(tc.tile_pool(name="php", bufs=4, space="PSUM"))
        const = ctx.enter_context(tc.tile_pool(name="const", bufs=1))
        xp = ctx.enter_context(tc.tile_pool(name="xp", bufs=NTD))
        op = ctx.enter_context(tc.tile_pool(name="op", bufs=3))
        up = ctx.enter_context(tc.tile_pool(name="up", bufs=4))
        hp = ctx.enter_context(tc.tile_pool(name="hp", bufs=4))
        gp = ctx.enter_context(tc.tile_pool(name="gp", bufs=6))
        cp = ctx.enter_context(tc.tile_pool(name="cp", bufs=1))

        # DRAM views with channel blocks split out: c = cb*128 + p.
        xv = x.rearrange("b (cb p) t -> b p cb t", p=P)
        ov = out.rearrange("b (cb p) t -> b p cb t", p=P)
        ovt = out.rearrange("b (cb p) t -> p cb b t", p=P)

        # Load order tuned for the pipeline fill: the first x window
        # (split per-batch so the transfers parallelize across DMA
        # queues) and the mm1 weights go first, with as few sync-queue
        # dispatches as possible — each dma_start costs the sync
        # sequencer ~750 ns, and compute can't start until these land.
        xtiles = {}
        xt = xp.tile([P, NCB, B_LOC, TCD], F16, tag="x", name="x0")
        for b in range(B_LOC):
            for cb in range(NCB):
                nc.sync.dma_start(xt[:, cb, b, :], xv[b, :, cb, 0:TCD])
        xtiles[0] = xt
        # w1z is already [p, cb*2P] on the host, so this is one flat DMA.
        w1all = const.tile([P, NCB * 2 * P], F16, tag="w1")
        nc.sync.dma_start(w1all[:], w1z[:])
        w1t = {
            (b, cb): w1all[:, cb * 2 * P + b * P:cb * 2 * P + (b + 1) * P]
            for b in range(B_LOC) for cb in range(NCB)
        }
        w2t = const.tile([P, 2 * P], F16, tag="w2")
        nc.sync.dma_start(w2t[:], w2z[:])
        b1t = const.tile([P, 1], F32, tag="b1")
        nc.sync.dma_start(b1t[:], b1z[:])
        b2t = const.tile([P, NCB], F32, tag="b2")
        nc.sync.dma_start(b2t[:], b2z[:])
        dconst = const.tile([P, TSC], F32, tag="dconst")
        nc.vector.memset(dconst[:], d)
        # Remaining x windows, all issued upfront (8 MB total fits
        # SBUF) so the DMA engines stream back-to-back.
        for td in range(1, NTD):
            xt = xp.tile([P, NCB, B_LOC, TCD], F16, tag="x", name=f"x{td}")
            for b in range(B_LOC):
                nc.sync.dma_start(
                    xt[:, :, b, :], xv[b, :, :, td * TCD:(td + 1) * TCD])
            xtiles[td] = xt

        def mm1(s):
            # p4 = [p_b0; p_b0; p_b1; p_b1] over a TSC-wide window. A
            # matmul output must fit one PSUM bank (512 f32), so the
            # 2-bank ph tile is filled by two interleaved 8-matmul
            # groups (one per bank — has_written bits are per bank).
            # Adjacent matmuls share the same weights.
            td, off = divmod(s, SPC)
            off *= TSC
            ph = php.tile([P, TSC], F32, tag="ph", name=f"ph{s}")
            ngrp = TSC // TS
            ntot = 2 * NCB * ngrp
            k = 0
            for cb in range(NCB):
                for b in range(B_LOC):
                    for hf in range(ngrp):
                        o = off + hf * TS
                        nc.tensor.matmul(
                            ph[:, hf * TS:(hf + 1) * TS], w1t[b, cb],
                            xtiles[td][:, cb, b, o:o + TS],
                            start=(k < ngrp), stop=(k >= ntot - ngrp))
                        k += 1
            return ph

        # Software-pipelined main loop. Per iteration sc we emit, in order:
        #   - mm1 for sc+1 (PE queue: ahead of sc's mm2s, so the scan
        #     feed never stalls on PE head-of-line blocking)
        #   - scan + relu for sc+1 (DVE queue: ahead of sc's gate-muls,
        #     which block on ACT sigmoids — otherwise the scan chain
        #     inherits the whole sigmoid cadence each chunk)
        #   - mm2 / sigmoid / gate-mul block for sc
        def scan_relu(sc, ph):
            if sc == 0:
                init = cp.tile([P, 1], F32, tag="c")
                nc.vector.tensor_scalar_mul(init[:], ph[:, 0:1], float(cw))
                init_ap = init[:]
            else:
                init_ap = uts[sc - 1][:, TSC - 1:TSC]
            ut = up.tile([P, TSC], F16, tag="u", name=f"u{sc}")
            nc.vector.tensor_tensor_scan(
                ut[:], dconst[:], ph[:], init_ap,
                mybir.AluOpType.mult, mybir.AluOpType.add)
            uts[sc] = ut
            ht = hp.tile([P, TSC], F16, tag="h", name=f"h{sc}")
            nc.vector.tensor_scalar(
                ht[:], ut[:], b1t[:], 0.0,
                mybir.AluOpType.add, mybir.AluOpType.max)
            return ht

        otiles = {}
        uts = {}
        hts = {}
        phs = {}
        phs[0] = mm1(0)
        hts[0] = scan_relu(0, phs[0])
        phs[1] = mm1(1)
        for sc in range(NSC):
            td = sc * TSC // TCD
            coff = sc * TSC - td * TCD
            if coff == 0:
                otiles[td] = op.tile(
                    [P, NCB, B_LOC, TCD], F16, tag="o", name=f"o{td}")
            xt = xtiles[td]
            ot = otiles[td]

            if sc + 1 < NSC:
                hts[sc + 1] = scan_relu(sc + 1, phs[sc + 1])
            ht = hts.pop(sc)

            # Per (half, channel block): row-tiled mm2 pair -> 2-bank PSUM
            # region, one sigmoid over [b0|b1], then two gate multiplies.
            for hf in range(TSC // TS):
                hoff = hf * TS
                off = coff + hoff
                for cb in range(NCB):
                    pg = pgp.tile([P, 2 * TS], F32, tag="pg")
                    rg0 = cb & 1
                    rg1 = 2 + (cb & 1)
                    half = cb >> 1
                    nc.tensor.matmul(
                        pg[:, 0:TS],
                        w2t[32 * rg0:32 * rg0 + 32, P * half:P * (half + 1)],
                        ht[32 * rg0:32 * rg0 + 32, hoff:hoff + TS],
                        start=True, stop=True, tile_position=(32 * rg0, 0))
                    nc.tensor.matmul(
                        pg[:, TS:2 * TS],
                        w2t[32 * rg1:32 * rg1 + 32, P * half:P * (half + 1)],
                        ht[32 * rg1:32 * rg1 + 32, hoff:hoff + TS],
                        start=True, stop=True, tile_position=(32 * rg1, 0))
                    gt = gp.tile([P, 2 * TS], F16, tag="g")
                    nc.scalar.activation(
                        gt[:], pg[:], mybir.ActivationFunctionType.Sigmoid,
                        bias=b2t[:, cb:cb + 1])
                    # All gate-muls on the DVE: gpsimd shares the DVE's
                    # SBUF port; a concurrent gpsimd op slows DVE ~3x.
                    for b in range(B_LOC):
                        nc.vector.tensor_mul(
                            ot[:, cb, b, off:off + TS],
                            xt[:, cb, b, off:off + TS],
                            gt[:, TS * b:TS * (b + 1)])
                    if sc == NSC - 1 and hf == TSC // TS - 1:
                        # Tail: store per cb as each mul lands.
                        nc.sync.dma_start(
                            ovt[:, cb, :, td * TCD:(td + 1) * TCD],
                            ot[:, cb, :, :])

            if coff + TSC == TCD and td != NTD - 1:
                for b in range(B_LOC):
                    nc.sync.dma_start(
                        ov[b, :, :, td * TCD:(td + 1) * TCD], ot[:, :, b, :])

            # PE filler emitted AFTER this chunk's mm2s: keeps a matmul
            # backlog in the PE queue so micro-idles (waiting on sigmoid
            # to free a pg bank) don't let HAM re-throttle the PE clock.
            if sc + 2 < NSC:
                phs[sc + 2] = mm1(sc + 2)
    nc.compile()
    return nc


def make_in_maps(x, w1, b1, w2, b2, cw, n_cores=N_CORES):
    """Host-side shard + weight prep. Returns per-core input maps."""
    a = 1.0 / cw
    w1s16 = np.ascontiguousarray((w1 * a).T).astype(np.float16)    # [C, CSQ]
    w1zz = np.zeros((C, 2 * P), dtype=np.float16)
    # b0 matmul fills out-partitions 0-63 (two copies), b1 fills 64-127.
    w1zz[:, 0:CSQ] = w1s16
    w1zz[:, CSQ:2 * CSQ] = w1s16
    w1zz[:, P + 2 * CSQ:P + 3 * CSQ] = w1s16
    w1zz[:, P + 3 * CSQ:P + 4 * CSQ] = w1s16
    # Device layout [p, cb*2P + m] so the weight load is one flat DMA.
    w1zz = np.ascontiguousarray(
        w1zz.reshape(NCB, P, 2 * P).transpose(1, 0, 2).reshape(P, -1))
    w2T16 = np.ascontiguousarray(w2.T).astype(np.float16)          # [CSQ, C]
    w2zz = np.zeros((P, 2 * P), dtype=np.float16)
    for rg in range(4):
        for hf in range(2):
            cb = 2 * hf + (rg & 1)
            w2zz[CSQ * rg:CSQ * (rg + 1), P * hf:P * (hf + 1)] = \
                w2T16[:, cb * P:(cb + 1) * P]
    b1zz = np.ascontiguousarray(
        np.tile(np.asarray(b1, np.float32), 4).reshape(P, 1))
    b2zz = np.ascontiguousarray(
        np.asarray(b2, np.float32).reshape(NCB, P).T)
    b_loc = x.shape[0] // n_cores
    x16 = np.asarray(x).astype(np.float16)
    return [
        {
            "x": np.ascontiguousarray(x16[i * b_loc:(i + 1) * b_loc]),
            "w1z": w1zz, "w2z": w2zz, "b1z": b1zz, "b2z": b2zz,
        }
        for i in range(n_cores)
    ]


_NC_CACHE = {}


def kernel(x, w1, b1, w2, b2, context_window):
    cw = int(context_window)
    x = np.asarray(x)
    key = (cw, x.shape)
    if key not in _NC_CACHE:
        _NC_CACHE[key] = build_nc(x.shape[0] // N_CORES, cw)
    nc = _NC_CACHE[key]
    in_maps = make_in_maps(
        x, np.asarray(w1), np.asarray(b1), np.asarray(w2), np.asarray(b2), cw)
    res = run_bass_kernel_spmd(nc, in_maps, core_ids=list(range(N_CORES)))
    return np.concatenate(
        [r["out"] for r in res.results], axis=0).astype(np.float32)
